# revision 13
# baseline (speedup 1.0000x reference)
"""Trainium2 Bass kernel for nn_Attention_81956565942967.

Cross-attention with key-length masking:
  B=8, N=1024, DIM=1024, HEADS=16, DIM_HEAD=64.

Sharding: pure data parallel — batch element b -> NeuronCore b. No
collectives. Host-side prep per shard: transpose x/context (so the
contraction dim lands on SBUF partitions) and cast the big operands to
bf16; compute the key-mask bias from lengths.

Device algorithm (per core, T-layout: features on partitions):
  qT = Wq^T xT          kT = Wk^T cT          v = (cT)^T Wv   (natural)
  per head h, query tile i:
    dotsT[j,i] = kT_h^T  qT_h            (K=64 matmuls)
    expT = Exp(SCALE*dotsT + maskbias_j)  (mask fused into ACT bias)
    o_unnorm[e,i] (+ rowsum at e=64) = [v_h | 1]^T expT
    catT_h = o_unnorm * (1/rowsum)       (PE outer-product broadcast)
  out = catT^T Wout + bout
"""

from contextlib import ExitStack

import ml_dtypes
import numpy as np

import concourse.bass as bass
from concourse import bacc
import concourse.mybir as mybir
import concourse.tile as tile
from concourse.bass_utils import run_bass_kernel_spmd

B, N, DIM = 8, 1024, 1024
HEADS, DIM_HEAD = 16, 64
INNER = HEADS * DIM_HEAD
SCALE = DIM_HEAD ** -0.5

P = 128
NT = N // P      # 8 partition tiles along n/j
KT = DIM // P    # 8 contraction tiles along dim/inner
FI = 512         # free-dim tile (PSUM bank)
NI = N // FI     # 2 query tiles
VW = 80  # v block per head: 64 dims + mask col at 64+h -> rowsum lands on psum row 64+h

BF = mybir.dt.bfloat16
F32 = mybir.dt.float32

_CACHE: dict = {}


def _build() -> bass.Bass:
    nc = bacc.Bacc("TRN2")

    xT_d = nc.dram_tensor("xT", [DIM, N], BF, kind="ExternalInput").ap()
    cT_d = nc.dram_tensor("cT", [DIM, N], BF, kind="ExternalInput").ap()
    wq_d = nc.dram_tensor("Wq", [DIM, INNER], BF, kind="ExternalInput").ap()
    wkv_d = nc.dram_tensor("Wkv", [DIM, 2 * INNER], BF, kind="ExternalInput").ap()
    wout_d = nc.dram_tensor("Wout", [INNER, DIM], BF, kind="ExternalInput").ap()
    bout_d = nc.dram_tensor("bout", [1, DIM], F32, kind="ExternalInput").ap()
    mask_d = nc.dram_tensor("maskb", [P, NT], F32, kind="ExternalInput").ap()
    sel_d = nc.dram_tensor("sel", [80, (HEADS // 2) * P], F32, kind="ExternalInput").ap()
    out_d = nc.dram_tensor("out", [N, DIM], F32, kind="ExternalOutput").ap()

    with tile.TileContext(nc) as tc, ExitStack() as ctx:
        const_p = ctx.enter_context(tc.tile_pool(name="const", bufs=1))
        exp_p = ctx.enter_context(tc.tile_pool(name="expp", bufs=2))
        stage_p = ctx.enter_context(tc.tile_pool(name="stage", bufs=3))
        acc_ps = ctx.enter_context(tc.tile_pool(name="accps", bufs=2, space="PSUM"))
        dots_ps = ctx.enter_context(tc.tile_pool(name="dotsps", bufs=3, space="PSUM"))
        po_ps = ctx.enter_context(tc.tile_pool(name="pops", bufs=2, space="PSUM"))
        pb_ps = ctx.enter_context(tc.tile_pool(name="pbps", bufs=1, space="PSUM"))

        # --- small constants ---
        mask_sb = const_p.tile([P, NT], F32, tag="mask")  # 1.0 valid / 0.0 masked
        nc.sync.dma_start(out=mask_sb, in_=mask_d)
        # selector for the pair-broadcast matmul: for pair pt,
        # sel[64+2pt, pt*128:pt*128+64] = 1 ; sel[64+2pt+1, pt*128+64:+64] = 1
        sel_sb = const_p.tile([80, (HEADS // 2) * P], F32, tag="sel")
        nc.sync.dma_start(out=sel_sb, in_=sel_d)
        rs_all = const_p.tile([80, N], F32, tag="rsall")
        rr_all = const_p.tile([80, N], F32, tag="rrall")
        bias_sb = const_p.tile([P, DIM], F32, tag="bias")
        nc.sync.dma_start(
            out=bias_sb,
            in_=bass.AP(tensor=bout_d.tensor, offset=bout_d.offset,
                        ap=[[0, P], [1, DIM]]),
        )

        # --- static SBUF tensors ---
        xT_t = [const_p.tile([P, N], BF, tag=f"xslot{t}", name=f"xs{t}") for t in range(KT)]
        cT_t = [const_p.tile([P, N], BF, tag=f"cslot{t}", name=f"cs{t}") for t in range(KT)]
        wq_t = [const_p.tile([P, INNER], BF, tag=f"wq{t}", name=f"wq{t}") for t in range(KT)]
        wkv_t = [const_p.tile([P, 2 * INNER], BF, tag=f"wkv{t}", name=f"wkv{t}") for t in range(KT)]
        wout_t = [const_p.tile([P, DIM], BF, tag=f"cslot{t}", name=f"wo{t}") for t in range(KT)]
        qT_t = [const_p.tile([P, N], BF, tag=f"q{t}", name=f"qt{t}") for t in range(KT)]
        kT_t = [const_p.tile([P, N], BF, tag=f"k{t}", name=f"kt{t}") for t in range(KT)]
        v_t = [const_p.tile([P, HEADS * VW], BF, tag=f"v{t}", name=f"vt{t}") for t in range(NT)]

        # v-projection inputs first so PE work starts ASAP
        for t in range(KT):
            nc.sync.dma_start(out=cT_t[t], in_=cT_d[t * P:(t + 1) * P, :])
            nc.sync.dma_start(out=wkv_t[t][:, INNER:],
                              in_=wkv_d[t * P:(t + 1) * P, INNER:])
        for t in range(KT):
            nc.sync.dma_start(out=wkv_t[t][:, :INNER],
                              in_=wkv_d[t * P:(t + 1) * P, :INNER])
        for t in range(KT):
            nc.sync.dma_start(out=xT_t[t], in_=xT_d[t * P:(t + 1) * P, :])
            nc.sync.dma_start(out=wq_t[t], in_=wq_d[t * P:(t + 1) * P, :])
        for t in range(NT):
            nc.vector.memset(v_t[t], 0.0)

        # --- projection helpers ---
        def proj_q(m):
            for i in range(NI):
                ps = acc_ps.tile([P, FI], F32, tag="acc", name="ps")
                for k in range(KT):
                    nc.tensor.matmul(
                        ps,
                        wq_t[k][:, m * P:(m + 1) * P],
                        xT_t[k][:, i * FI:(i + 1) * FI],
                        start=(k == 0), stop=(k == KT - 1),
                    )
                nc.vector.tensor_copy(qT_t[m][:, i * FI:(i + 1) * FI], ps)

        def proj_k(m):
            for i in range(NI):
                ps = acc_ps.tile([P, FI], F32, tag="acc", name="ps")
                for k in range(KT):
                    nc.tensor.matmul(
                        ps,
                        wkv_t[k][:, m * P:(m + 1) * P],
                        cT_t[k][:, i * FI:(i + 1) * FI],
                        start=(k == 0), stop=(k == KT - 1),
                    )
                nc.vector.tensor_copy(kT_t[m][:, i * FI:(i + 1) * FI], ps)

        def proj_v(t):
            v3 = v_t[t].rearrange("p (h w) -> p h w", w=VW)
            for i2 in range(NI):
                ps = acc_ps.tile([P, FI], F32, tag="acc", name="ps")
                for k in range(KT):
                    nc.tensor.matmul(
                        ps,
                        cT_t[k][:, t * P:(t + 1) * P],
                        wkv_t[k][:, INNER + i2 * FI:INNER + (i2 + 1) * FI],
                        start=(k == 0), stop=(k == KT - 1),
                    )
                nc.vector.tensor_copy(
                    v3[:, i2 * 8:(i2 + 1) * 8, 0:DIM_HEAD],
                    ps.rearrange("p (h d) -> p h d", d=DIM_HEAD),
                )
            # mask col of head h sits at flat position h*VW + 64 + h = 64 + 81*h
            diag = bass.AP(tensor=v_t[t].tensor, offset=v_t[t].offset + DIM_HEAD,
                           ap=[list(v_t[t].ap[0]), [VW + 1, HEADS]])
            nc.vector.tensor_scalar_mul(
                diag, mask_sb[:, t:t + 1].to_broadcast([P, HEADS]), 1.0)

        catT_t = [const_p.tile([P, N], BF, tag=f"xslot{t}", name=f"cat{t}") for t in range(KT)]

        def final_tile(t):
            for i in range(NI):
                pf = acc_ps.tile([P, FI], F32, tag="acc", name="pf")
                for k in range(KT):
                    nc.tensor.matmul(
                        pf,
                        catT_t[k][:, t * P:(t + 1) * P],
                        wout_t[k][:, i * FI:(i + 1) * FI],
                        start=(k == 0), stop=(k == KT - 1),
                    )
                ot = stage_p.tile([P, FI], F32, tag="ot", name="ot")
                nc.vector.tensor_tensor(
                    ot, pf, bias_sb[:, i * FI:(i + 1) * FI],
                    mybir.AluOpType.add,
                )
                nc.sync.dma_start(
                    out=out_d[t * P:(t + 1) * P, i * FI:(i + 1) * FI], in_=ot,
                )

        def attention(i, hp):
            isl = slice(i * FI, (i + 1) * FI)
            pt = hp
            ets = []
            for sub in range(2):
                ets.append(exp_p.tile([P, NT, FI], BF, tag=f"exp{sub}",
                                      name=f"et{sub}"))
            # paired dots: heads 2hp (partitions 0:64) and 2hp+1 (64:128)
            # occupy different PE row groups -> run concurrently
            for j in range(NT):
                for sub in range(2):
                    off = sub * DIM_HEAD
                    dps = dots_ps.tile([P, FI], F32, tag="dots", name="dps")
                    nc.tensor.matmul(
                        dps,
                        kT_t[pt][off:off + DIM_HEAD, j * P:(j + 1) * P],
                        qT_t[pt][off:off + DIM_HEAD, isl],
                        start=True, stop=True,
                    )
                    nc.scalar.activation(
                        ets[sub][:, j, :], dps,
                        mybir.ActivationFunctionType.Exp, scale=SCALE,
                    )
            for sub in range(2):
                h = 2 * hp + sub
                off = sub * DIM_HEAD
                et = ets[sub]
                po = po_ps.tile([VW, FI], F32, tag="po", name="po")
                for j in range(NT):
                    nc.tensor.matmul(
                        po,
                        v_t[j][:, h * VW:(h + 1) * VW],
                        et[:, j, :],
                        start=(j == 0), stop=(j == NT - 1),
                    )
                cslice = catT_t[pt][off:off + DIM_HEAD, isl]
                nc.vector.tensor_copy(cslice, po[0:DIM_HEAD, :])
                nc.vector.tensor_tensor(
                    rs_all[64:80, isl], rs_all[64:80, isl], po[64:80, :],
                    mybir.AluOpType.add)

        def norm(i):
            isl = slice(i * FI, (i + 1) * FI)
            nc.vector.reciprocal(rr_all[64:80, isl], rs_all[64:80, isl])
            for pt in range(HEADS // 2):
                pb = pb_ps.tile([P, FI], F32, tag="pb", name="pb")
                nc.tensor.matmul(
                    pb,
                    sel_sb[64:80, pt * P:(pt + 1) * P],
                    rr_all[64:80, isl],
                    start=True, stop=True,
                )
                nc.vector.tensor_tensor(
                    catT_t[pt][:, isl], catT_t[pt][:, isl], pb,
                    mybir.AluOpType.mult,
                )

        # --- schedule: v, k first; qT tile 0; then attention with PE
        # gap-filler work interleaved (qT projections during i=0,
        # final projections during i=1) to keep the PE HAM-warm ---
        for t in range(NT):
            proj_v(t)
        for m in range(KT):
            proj_k(m)
        for t in range(KT):
            nc.sync.dma_start(out=wout_t[t], in_=wout_d[t * P:(t + 1) * P, :])
        proj_q(0)

        nc.vector.memset(rs_all[64:80, 0:FI], 0.0)
        for hp in range(HEADS // 2):
            attention(0, hp)
            if hp + 1 < KT:
                proj_q(hp + 1)
        norm(0)
        nc.vector.memset(rs_all[64:80, FI:N], 0.0)
        for hp in range(HEADS // 2):
            attention(1, hp)
            if hp < 4:
                final_tile(hp)
        norm(1)
        for t in range(4, NT):
            final_tile(t)

    nc.finalize()
    return nc


def _prep_in_maps(x, context, lengths, Wq, Wkv, Wout, bout):
    bf = ml_dtypes.bfloat16
    wq = np.ascontiguousarray(Wq, dtype=bf)
    wkv = np.ascontiguousarray(Wkv, dtype=bf)
    wout = np.ascontiguousarray(Wout, dtype=bf)
    bo = np.ascontiguousarray(np.asarray(bout, dtype=np.float32).reshape(1, DIM))
    jj = np.arange(N).reshape(NT, P)  # [j_tile, partition]
    sel = np.zeros((80, (HEADS // 2) * P), dtype=np.float32)
    for pt in range(HEADS // 2):
        sel[64 + 2 * pt, pt * P:pt * P + DIM_HEAD] = 1.0
        sel[65 + 2 * pt, pt * P + DIM_HEAD:(pt + 1) * P] = 1.0
    in_maps = []
    context = np.asarray(context)
    for b in range(B):
        mb = np.where(jj < int(lengths[b]), 1.0, 0.0).astype(np.float32)
        cb = context[b].copy()
        cb[int(lengths[b]):] = 0.0
        in_maps.append({
            "xT": np.ascontiguousarray(np.asarray(x[b]).T, dtype=bf),
            "cT": np.ascontiguousarray(cb.T, dtype=bf),
            "Wq": wq, "Wkv": wkv, "Wout": wout, "bout": bo,
            "maskb": np.ascontiguousarray(mb.T), "sel": sel,
        })
    return in_maps


def run(inputs: dict, trace: bool = False):
    if "nc" not in _CACHE:
        _CACHE["nc"] = _build()
    nc = _CACHE["nc"]
    in_maps = _prep_in_maps(**inputs)
    res = run_bass_kernel_spmd(nc, in_maps, core_ids=list(range(B)), trace=trace)
    out = np.stack([res.results[i]["out"] for i in range(B)]).astype(np.float32)
    return out, res


def kernel(**inputs) -> np.ndarray:
    out, _ = run(inputs, trace=False)
    return out
